# revision 4
# baseline (speedup 1.0000x reference)
"""Trainium2 Bass kernel for nn_ActionReselector (topk_masking).

reference:
    q = city_embed @ Wq                 [B, NC, D]
    k = agent_embed @ Wk                [B, NA, D]
    scores = q @ k.T / sqrt(D)          [B, NC, NA]
    out = argmax(10*tanh(scores), -1)   [B, NC] int32

Key identities used here:
  - tanh and the positive scales are strictly monotonic -> argmax(scores).
  - scores = city @ (Wq @ k.T) = city @ M with M = [D, NA] tiny.
So per batch we compute M once (three small matmuls), then stream city
through PE (transpose via identity matmul, then scores matmul), and do the
agent-argmax with the DVE max/max_index ops (slot 0 = first index of max,
matching jnp.argmax tie-break).

Sharding: data-parallel over batch B=64 across 8 cores (8 batches/core).
"""

import numpy as np

import concourse.bacc as bacc
import concourse.bass as bass
import concourse.mybir as mybir
import concourse.tile as tile
from concourse import masks

# Problem shapes (hardcoded per contract)
B = 64
NA = 100
NC = 5000
D = 128
N_CORES = 8
B_PER_CORE = B // N_CORES

# City tiling: chunks of 500 cities DMA'd as [125 partitions x 4 cities x 128]
# so each partition line is 2KB contiguous in DRAM.
CHUNK = 500
CPP = 4            # cities per partition within a chunk
P_USED = CHUNK // CPP   # 125
NCHUNK = NC // CHUNK    # 10
NSUB = NCHUNK * CPP     # 40 sub-blocks (each 125 cities) per batch

F32 = mybir.dt.float32
U32 = mybir.dt.uint32


def build_nc():
    nc = bacc.Bacc(None, target_bir_lowering=False)

    city = nc.dram_tensor("city", [B_PER_CORE, NC, D], F32, kind="ExternalInput")
    agent = nc.dram_tensor("agent", [B_PER_CORE, NA, D], F32, kind="ExternalInput")
    wq = nc.dram_tensor("wq", [D, D], F32, kind="ExternalInput")
    wk = nc.dram_tensor("wk", [D, D], F32, kind="ExternalInput")
    out = nc.dram_tensor("out", [B_PER_CORE, P_USED, NSUB], U32, kind="ExternalOutput")

    with tile.TileContext(nc) as tc:
        with (
            tc.tile_pool(name="const", bufs=1) as constp,
            tc.tile_pool(name="weights", bufs=1) as wp,
            tc.tile_pool(name="cityin", bufs=3) as cityp,
            tc.tile_pool(name="cityT", bufs=3) as ctp,
            tc.tile_pool(name="psumT", bufs=3, space="PSUM") as ptp,
            tc.tile_pool(name="psumS", bufs=4, space="PSUM") as psp,
            tc.tile_pool(name="mmat", bufs=2) as mp,
            tc.tile_pool(name="small", bufs=3) as smallp,
            tc.tile_pool(name="stage", bufs=2) as stagep,
        ):
            ident = constp.tile([128, 128], F32)
            masks.make_identity(nc, ident[:])

            wq_sb = wp.tile([128, 128], F32)
            nc.sync.dma_start(wq_sb[:], wq[:])
            wk_sb = wp.tile([128, 128], F32)
            nc.sync.dma_start(wk_sb[:], wk[:])

            # WqT (one-time): transpose Wq so M = Wq @ kT = (WqT).T @ kT
            wqT_ps = ptp.tile([128, 128], F32, tag="pt")
            nc.tensor.transpose(wqT_ps[:], wq_sb[:], ident[:])
            wqT = wp.tile([128, 128], F32)
            nc.scalar.copy(wqT[:], wqT_ps[:])

            for b in range(B_PER_CORE):
                # ---- build M[d, a] = Wq @ k^T for this batch ----
                atile = smallp.tile([NA, D], F32, tag="agent")
                nc.sync.dma_start(atile[:], agent[b])

                aT_ps = ptp.tile([128, NA], F32, tag="pt")
                nc.tensor.transpose(aT_ps[:], atile[:], ident[:NA, :NA])
                aT = smallp.tile([128, NA], F32, tag="aT")
                nc.scalar.copy(aT[:], aT_ps[:])

                # kT[e, a] = sum_d Wk[d, e] * agentT[d, a]
                kT_ps = ptp.tile([128, NA], F32, tag="pt")
                nc.tensor.matmul(kT_ps[:], wk_sb[:], aT[:], start=True, stop=True)
                kT = smallp.tile([128, NA], F32, tag="kT")
                nc.scalar.copy(kT[:], kT_ps[:])

                # M[d, a] = sum_e WqT[e, d] * kT[e, a]
                m_ps = ptp.tile([128, NA], F32, tag="pt")
                nc.tensor.matmul(m_ps[:], wqT[:], kT[:], start=True, stop=True)
                msb = mp.tile([128, NA], F32)
                nc.scalar.copy(msb[:], m_ps[:])

                staging = stagep.tile([128, NSUB * 8], U32, tag="staging")
                stagc = stagep.tile([128, NSUB], U32, tag="stagc")

                for ch in range(NCHUNK):
                    ctile = cityp.tile([P_USED, CPP, D], F32)
                    src = city[b, ch * CHUNK:(ch + 1) * CHUNK, :]
                    nc.sync.dma_start(
                        ctile[:], src.rearrange("(p n) d -> p n d", n=CPP)
                    )
                    for n in range(CPP):
                        s = ch * CPP + n
                        # cityT block: [d=128, c=125]
                        ctT_ps = ptp.tile([D, P_USED], F32, tag="pt")
                        nc.tensor.transpose(
                            ctT_ps[:], ctile[:, n, :], ident[:P_USED, :P_USED]
                        )
                        ctTs = ctp.tile([D, P_USED], F32)
                        nc.scalar.copy(ctTs[:], ctT_ps[:])

                        # scores[c, a] = sum_d cityT[d, c] * M[d, a]
                        sc_ps = psp.tile([P_USED, NA], F32, tag="sc")
                        nc.tensor.matmul(
                            sc_ps[:], ctTs[:], msb[:], start=True, stop=True
                        )

                        maxv = smallp.tile([P_USED, 8], F32, tag="maxv")
                        nc.vector.max(maxv[:], sc_ps[:])
                        nc.vector.max_index(
                            staging[:P_USED, s * 8:(s + 1) * 8], maxv[:], sc_ps[:]
                        )

                # compact slot-0 indices [125, NSUB] and store
                nc.vector.tensor_copy(
                    stagc[:P_USED, :],
                    staging[:P_USED, :].rearrange("p (s e) -> p s e", e=8)[:, :, 0],
                )
                nc.sync.dma_start(out[b], stagc[:P_USED, :])

    nc.finalize()
    return nc


_RUNNER = None


class _Runner:
    """Compile the bass program once; allow repeated execution.

    Mirrors concourse.bass2jax.run_bass_via_pjrt's multi-core branch, but
    keeps the jitted sharded callable so repeat calls don't recompile.
    """

    def __init__(self):
        import jax
        from jax.experimental.shard_map import shard_map
        from jax.sharding import Mesh, NamedSharding, PartitionSpec

        import concourse.mybir as _mybir
        from concourse import bass2jax

        self.jax = jax
        self.NamedSharding = NamedSharding
        self.PartitionSpec = PartitionSpec

        bass2jax.install_neuronx_cc_hook()
        nc = build_nc()
        self.nc = nc
        assert nc.dbg_addr is None

        partition_name = (
            nc.partition_id_tensor.name if nc.partition_id_tensor else None
        )
        in_names, out_names, out_avals, zero_outs = [], [], [], []
        for alloc in nc.m.functions[0].allocations:
            if not isinstance(alloc, _mybir.MemoryLocationSet):
                continue
            name = alloc.memorylocations[0].name
            if alloc.kind == "ExternalInput":
                if name != partition_name:
                    in_names.append(name)
            elif alloc.kind == "ExternalOutput":
                shape = tuple(alloc.tensor_shape)
                dtype = _mybir.dt.np(alloc.dtype)
                out_names.append(name)
                out_avals.append(jax.core.ShapedArray(shape, dtype))
                zero_outs.append(np.zeros(shape, dtype))
        n_params = len(in_names)
        n_outs = len(out_avals)
        all_in_names = list(in_names) + list(out_names)
        if partition_name is not None:
            all_in_names.append(partition_name)

        self.in_names = in_names
        self.out_names = out_names
        self.out_avals = out_avals
        self.zero_outs = zero_outs
        self.n_params = n_params

        donate = tuple(range(n_params, n_params + n_outs))

        def _body(*args):
            operands = list(args)
            if partition_name is not None:
                operands.append(bass2jax.partition_id_tensor())
            outs = bass2jax._bass_exec_p.bind(
                *operands,
                out_avals=tuple(out_avals),
                in_names=tuple(all_in_names),
                out_names=tuple(out_names),
                lowering_input_output_aliases=(),
                sim_require_finite=True,
                sim_require_nnan=True,
                nc=nc,
            )
            return tuple(outs)

        devices = jax.devices()[:N_CORES]
        assert len(devices) == N_CORES
        self.mesh = Mesh(np.asarray(devices), ("core",))
        in_specs = (PartitionSpec("core"),) * (n_params + n_outs)
        out_specs = (PartitionSpec("core"),) * n_outs
        self.sharded = jax.jit(
            shard_map(
                _body,
                mesh=self.mesh,
                in_specs=in_specs,
                out_specs=out_specs,
                check_rep=False,
            ),
            donate_argnums=donate,
            keep_unused=True,
        )

    def concat_inputs(self, in_maps):
        return [
            np.concatenate(
                [np.asarray(m[name]) for m in in_maps], axis=0
            )
            for name in self.in_names
        ]

    def device_inputs(self, in_maps):
        """Pre-place concatenated inputs on the mesh (for timing loops)."""
        spec = self.NamedSharding(self.mesh, self.PartitionSpec("core"))
        return [
            self.jax.device_put(a, spec) for a in self.concat_inputs(in_maps)
        ]

    def concat_zeros(self):
        return [
            np.zeros((N_CORES * z.shape[0], *z.shape[1:]), z.dtype)
            for z in self.zero_outs
        ]

    def execute(self, placed_inputs):
        outs = self.sharded(*placed_inputs, *self.concat_zeros())
        self.jax.block_until_ready(outs)
        return outs

    def run(self, in_maps):
        out_arrs = self.execute(self.concat_inputs(in_maps))
        return [
            {
                name: np.asarray(out_arrs[i]).reshape(
                    N_CORES, *self.out_avals[i].shape
                )[c]
                for i, name in enumerate(self.out_names)
            }
            for c in range(N_CORES)
        ]


def _make_runner():
    global _RUNNER
    if _RUNNER is None:
        _RUNNER = _Runner()
    return _RUNNER


def _unshuffle(raw: np.ndarray) -> np.ndarray:
    """[B_PER_CORE, 125, 40] u32 -> [B_PER_CORE, 5000] city-ordered."""
    a = raw.reshape(B_PER_CORE, P_USED, NCHUNK, CPP)  # [b, p, ch, n]
    a = a.transpose(0, 2, 1, 3)                       # [b, ch, p, n]
    return a.reshape(B_PER_CORE, NC)                  # c = ch*500 + 4p + n


def kernel(agent_embed, city_embed, Wq, Wk):
    agent_embed = np.ascontiguousarray(np.asarray(agent_embed, dtype=np.float32))
    city_embed = np.ascontiguousarray(np.asarray(city_embed, dtype=np.float32))
    Wq = np.ascontiguousarray(np.asarray(Wq, dtype=np.float32))
    Wk = np.ascontiguousarray(np.asarray(Wk, dtype=np.float32))

    runner = _make_runner()
    in_maps = [
        {
            "city": city_embed[i * B_PER_CORE:(i + 1) * B_PER_CORE],
            "agent": agent_embed[i * B_PER_CORE:(i + 1) * B_PER_CORE],
            "wq": Wq,
            "wk": Wk,
        }
        for i in range(N_CORES)
    ]
    outs = runner.run(in_maps)
    full = np.empty((B, NC), dtype=np.int32)
    for i in range(N_CORES):
        full[i * B_PER_CORE:(i + 1) * B_PER_CORE] = _unshuffle(
            outs[i]["out"]
        ).astype(np.int32)
    return full
